# revision 1
# baseline (speedup 1.0000x reference)
"""PCEN (per-channel energy normalization) Trainium2 Bass kernel.

Problem: data [1024, 50000] f32, EMA along time (s=0.5) then
    out = (x / (EPS + M)**alpha + delta)**r - delta**r

Sharding: freq axis (dim 0) split across 8 NeuronCores, 128 rows/core.
Per core the EMA recurrence runs on the DVE's native tensor_tensor_scan
instruction (1 elem/lane/cycle): we compute M2_t = 0.5*M2_{t-1} + x_t
which equals 2*M_t bit-exactly (mult by 0.5 is exact in fp32, one
rounding per step, identical to the reference recurrence scaled by 2),
and fold the 0.5 into the ACT Ln scale.

Per-tile chain (TC time columns):
    scan: M2 = scan(0.5*state + x)                 [DVE]
    lnP  = Ln(0.5*M2 + EPS)                        [ACT, set natural_log_exp]
    e    = Exp(-alpha*lnP)                         [ACT, same set]
    R    = x*e                                     [DVE]
    S    = Sqrt(R + delta)   (r == 0.5)            [ACT, sqrt set]
    out  = S - delta**r                            [GpSimd]

ACT table sets: Ln+Exp share one set only in `natural_log_exp_and_others`;
Sqrt lives in its own set. Two measures keep ACT_TABLE_LOAD (~1.3-2.7us
each) off the critical path: (a) get_activation_tables is patched so the
greedy load-insertion pass can only satisfy Ln/Exp from the shared set
(instead of alternating natural_log <-> exp_and_others every call), and
(b) tiles are processed in groups of G: all Ln/Exp/mult for the group
first, then all Sqrt/sub/store, so the set switch cost amortizes over G
tiles (2 loads per G tiles instead of 2 per tile).
"""

import numpy as np

import concourse.bass as bass
import concourse.bacc as bacc
import concourse.mybir as mybir
from concourse import tile
from concourse.bass_utils import run_bass_kernel_spmd

F, T = 1024, 50000
NCORES = 8
FP = F // NCORES  # 128 partitions per core
TC = 2500         # main time-tile width (10 KB/partition, f32)
# ACT-table-set groups of tile widths. The two small leading tiles get
# the first Ln onto the ACT engine ~6us sooner (first DMA+scan are short);
# the small trailing group shortens the serialized drain.
GROUPS = (
    (500, 1000, 1500, 2000, TC, TC, TC),
    (TC,) * 6,
    (TC,) * 7,
    (TC,) * 2,
)
EPS = 1e-6

_CACHE: dict = {}

# Restrict Ln/Exp to the one table set that holds both, so the greedy
# ACT-table-load pass emits a single resident set for the Ln->Exp chain
# instead of thrashing between `natural_log` and `exp_and_others` on
# every activation. Only the pass's coverage analysis sees this dict;
# the emitted set genuinely contains both functions, so the loaded
# hardware tables are correct.
_orig_gat = bacc.get_activation_tables


def _patched_gat(arch):
    A = mybir.ActivationFunctionType
    out = {}
    for name, fns in _orig_gat(arch).items():
        fns = set(fns)
        if name != "natural_log_exp_and_others":
            fns.discard(A.Ln)
            fns.discard(A.Exp)
        out[name] = fns
    return out


bacc.get_activation_tables = _patched_gat


def _build(alpha: float, r: float, delta: float):
    dt = mybir.dt.float32
    Act = mybir.ActivationFunctionType
    Alu = mybir.AluOpType
    c = float(delta) ** float(r)
    use_sqrt = abs(r - 0.5) < 1e-12

    nc = bacc.Bacc("TRN2", debug=False, enable_asserts=False,
                   target_bir_lowering=False)
    x = nc.dram_tensor("x", [FP, T], dt, kind="ExternalInput").ap()
    y = nc.dram_tensor("y", [FP, T], dt, kind="ExternalOutput").ap()

    with tile.TileContext(nc) as tc:
        with (
            tc.tile_pool(name="const", bufs=1) as cpool,
            tc.tile_pool(name="x", bufs=6) as xpool,
            tc.tile_pool(name="m", bufs=4) as mpool,
            tc.tile_pool(name="l", bufs=10) as lpool,
        ):
            # stride-0 broadcast [FP,1] const: avoids a TC-wide memset on
            # the critical path to the first scan (verified bit-exact)
            half = cpool.tile([FP, 1], dt, tag="half")
            nc.gpsimd.memset(half[:], 0.5)
            eps_t = cpool.tile([FP, 1], dt, tag="eps")
            nc.gpsimd.memset(eps_t[:], EPS)
            delta_t = cpool.tile([FP, 1], dt, tag="delta")
            nc.gpsimd.memset(delta_t[:], float(delta))

            carry = 0.0
            acts = []  # ACT instructions in intended engine order
            # Warm-up activation with no data dependencies: the implicit
            # first ACT_TABLE_LOAD is inserted before it and runs during
            # the preamble instead of waiting behind the first scan.
            warm = cpool.tile([FP, 1], dt, tag="warm")
            acts.append(nc.scalar.activation(warm[:], eps_t[:], Act.Ln,
                                             bias=eps_t[:], scale=0.5))
            off = 0
            for gi, grp in enumerate(GROUPS):
                last_group = gi == len(GROUPS) - 1
                infos = []
                # phase A: load, scan, Ln, Exp, mult  (ln/exp table set)
                for w in grp:
                    xt = xpool.tile([FP, TC], dt, tag="x")
                    nc.sync.dma_start(xt[:, :w], x[:, off:off + w])
                    m2 = mpool.tile([FP, TC], dt, tag="m")
                    nc.vector.tensor_tensor_scan(
                        m2[:, :w], half[:].to_broadcast((FP, w)), xt[:, :w],
                        carry, Alu.mult, Alu.add)
                    carry = m2[:, w - 1:w]
                    lt = lpool.tile([FP, TC], dt, tag="l")
                    acts.append(nc.scalar.activation(lt[:, :w], m2[:, :w],
                                                     Act.Ln, bias=eps_t[:],
                                                     scale=0.5))
                    acts.append(nc.scalar.activation(lt[:, :w], lt[:, :w],
                                                     Act.Exp, scale=-alpha))
                    # first group's mults go to GpSimd (idle early) so the
                    # DVE runs the serial scan chain uninterrupted during
                    # pipeline ramp-up. The mult writes into lt, not xt: the
                    # x slot then frees at mult time, so the in-DMA/scan
                    # prefetch chain is decoupled from phase-B slot recycling
                    # (R rides in the l pool through sqrt/sub/store).
                    meng = nc.gpsimd if gi == 0 else nc.vector
                    meng.tensor_tensor(lt[:, :w], xt[:, :w], lt[:, :w],
                                       Alu.mult)
                    infos.append((lt, off, w))
                    off += w
                # phase B: power, subtract, store  (sqrt table set).
                # The last group drains serially after ACT's final work, so
                # chunk it finely and alternate its subs across DVE/GpSimd
                # (both idle by then) to pipeline sub+store behind the
                # sqrt chunks and shorten the kernel tail.
                ci = 0
                for xt, o, w in infos:
                    cw = 500 if last_group else w
                    lo = 0
                    while lo < w:
                        hi = min(lo + cw, w)
                        if use_sqrt:
                            acts.append(nc.scalar.activation(
                                xt[:, lo:hi], xt[:, lo:hi], Act.Sqrt,
                                bias=delta_t[:], scale=1.0))
                        else:
                            acts.append(nc.scalar.activation(
                                xt[:, lo:hi], xt[:, lo:hi], Act.Ln,
                                bias=delta_t[:], scale=1.0))
                            acts.append(nc.scalar.activation(
                                xt[:, lo:hi], xt[:, lo:hi], Act.Exp,
                                scale=float(r)))
                        eng = nc.vector if (last_group and ci % 2 == 0) \
                            else nc.gpsimd
                        eng.tensor_scalar_add(xt[:, lo:hi], xt[:, lo:hi], -c)
                        nc.sync.dma_start(y[:, o + lo:o + hi], xt[:, lo:hi])
                        lo = hi
                        ci += 1
            # Pin the ACT stream to program order so phase-A/phase-B
            # batching survives the scheduler's gap-filling — otherwise a
            # ready Sqrt slips between Ln/Exp pairs and every slip costs an
            # ACT_TABLE_LOAD set switch.
            for prev, nxt in zip(acts, acts[1:]):
                tile.add_dep_helper(nxt.ins, prev.ins, sync=False,
                                    reason="ACT table-set batching order")

    nc.compile()
    return nc


def _get_nc(alpha: float, r: float, delta: float):
    key = (round(alpha, 9), round(r, 9), round(delta, 9))
    if key not in _CACHE:
        _CACHE[key] = _build(alpha, r, delta)
    return _CACHE[key]


def _make_runner(nc):
    """Cached variant of bass2jax.run_bass_via_pjrt's multi-core branch.

    run_bass_kernel_spmd builds a fresh jax.jit closure per call (full
    retrace) and round-trips the full array through per-core split +
    concat. Since the 8 shards concatenated on axis 0 ARE the full
    [1024, 50000] array, we jit once and feed/return the full array
    directly.
    """
    import jax
    from jax.experimental.shard_map import shard_map
    from jax.sharding import Mesh, PartitionSpec
    from concourse import bass2jax

    bass2jax.install_neuronx_cc_hook()
    if nc.dbg_callbacks:
        raise RuntimeError("dbg callbacks unsupported in cached runner")
    partition_name = (nc.partition_id_tensor.name
                      if nc.partition_id_tensor else None)
    in_names, out_names, out_avals = [], [], []
    for alloc in nc.m.functions[0].allocations:
        if not isinstance(alloc, mybir.MemoryLocationSet):
            continue
        name = alloc.memorylocations[0].name
        if alloc.kind == "ExternalInput":
            if name != partition_name:
                in_names.append(name)
        elif alloc.kind == "ExternalOutput":
            out_names.append(name)
            out_avals.append(jax.core.ShapedArray(
                tuple(alloc.tensor_shape), mybir.dt.np(alloc.dtype)))
    extra_ins = {}
    if nc.dbg_addr is not None:
        extra_ins[nc.dbg_addr.name] = np.zeros((1, 2), np.uint32)
        if nc.dbg_addr.name not in in_names:
            in_names.append(nc.dbg_addr.name)
    assert in_names[0] == "x" and out_names == ["y"], (in_names, out_names)
    n_params = len(in_names)
    all_names = list(in_names) + list(out_names)
    if partition_name is not None:
        all_names.append(partition_name)
    donate = tuple(range(n_params, n_params + len(out_names)))

    def _body(*args):
        operands = list(args)
        if partition_name is not None:
            operands.append(bass2jax.partition_id_tensor())
        outs = bass2jax._bass_exec_p.bind(
            *operands,
            out_avals=tuple(out_avals),
            in_names=tuple(all_names),
            out_names=tuple(out_names),
            lowering_input_output_aliases=(),
            sim_require_finite=True,
            sim_require_nnan=True,
            nc=nc,
        )
        return tuple(outs)

    devices = jax.devices()[:NCORES]
    assert len(devices) == NCORES, devices
    mesh = Mesh(np.asarray(devices), ("core",))
    nio = n_params + len(out_names)
    sharded = jax.jit(
        shard_map(_body, mesh=mesh,
                  in_specs=(PartitionSpec("core"),) * nio,
                  out_specs=(PartitionSpec("core"),) * len(out_names),
                  check_rep=False),
        donate_argnums=donate, keep_unused=True)

    def run(data: np.ndarray) -> np.ndarray:
        extras = [np.concatenate([v] * NCORES, axis=0)
                  for v in extra_ins.values()]
        zeros = [np.zeros((NCORES * a.shape[0], *a.shape[1:]), a.dtype)
                 for a in out_avals]
        outs = sharded(data, *extras, *zeros)
        return np.asarray(outs[0])

    return run


def kernel(data, alpha=None, r=None, delta=None) -> np.ndarray:
    data = np.ascontiguousarray(np.asarray(data, dtype=np.float32))
    assert data.shape == (F, T), data.shape
    a = float(np.asarray(alpha).reshape(-1)[0]) if alpha is not None else 0.98
    rr = float(np.asarray(r).reshape(-1)[0]) if r is not None else 0.5
    d = float(np.asarray(delta).reshape(-1)[0]) if delta is not None else 2.0

    nc = _get_nc(a, rr, d)
    rkey = ("runner", round(a, 9), round(rr, 9), round(d, 9))
    try:
        if rkey not in _CACHE:
            _CACHE[rkey] = _make_runner(nc)
        return _CACHE[rkey](data)
    except Exception:  # fall back to the stock SPMD path
        _CACHE[rkey] = None
        in_maps = [{"x": data[i * FP:(i + 1) * FP]} for i in range(NCORES)]
        res = run_bass_kernel_spmd(nc, in_maps, core_ids=list(range(NCORES)))
        return np.concatenate([res.results[i]["y"] for i in range(NCORES)],
                              axis=0)



# revision 4
# speedup vs baseline: 90395.9076x; 90395.9076x over previous
"""PCEN (per-channel energy normalization) Trainium2 Bass kernel, v2.

Problem: data [1024, 50000] f32, EMA along time (s=0.5) then
    out = (x / (EPS + M)**alpha + delta)**r - delta**r

Sharding: freq axis (dim 0) split across 8 NeuronCores, 128 rows/core.

v2 engine plan (per core, 50000 cols x 128 lanes):
  DMA   in/out fp16 (host converts f32<->fp16)        ~35.5us each way
  Pool  EMA scan  M2 = scan(0.5*state + xs)           ~69us (0.6 eff)
  ACT   L  = Ln(M2*(0.5/32) + EPS)        fp32 out    ~42us
  ACT   e  = Exp(-alpha*L - K - ln 32)    fp16 out    ~42us  <- critical
  DVE   R  = xs*e                         fp16 2x     ~26us
  DVE   u  = R + a                        fp16 4x     ~13us
  DVE   u2 = u*R                          fp16 2x     ~26us
  DVE   y  = u2*c2s + c0                  fp16 4x     ~13us

The three ACT passes of the v1 kernel (Ln, Exp, Sqrt) are cut to two:
since x_t <= M2_t (the scan dominates its input), R = x*(EPS+M)^-alpha
lies in [0, 2], so sqrt(R+delta)-delta**r is replaced by a degree-2
minimax polynomial evaluated on the DVE (max err ~2e-3, tolerance 2e-2).

Range management, all folded into existing constants (zero extra ops):
  - host sends xs = x*32 in fp16, lifting tiny x out of fp16-subnormal
    loss; the Ln scale absorbs the /32.
  - the Exp bias shifts e by e^-(K+ln32) so e stays in fp16 range for
    any P >= EPS (max ~160); the poly coefficients absorb e^K back.

fp16 tensors are only those the DVE/DMA touch (xs, e/R, u, out); the
scan state, M2 and L stay fp32 so no precision is lost there.
"""

import numpy as np

import concourse.bass as bass
import concourse.bacc as bacc
import concourse.mybir as mybir
from concourse import tile
from concourse.bass_utils import run_bass_kernel_spmd

F, T = 1024, 50000
NCORES = 8
FP = F // NCORES  # 128 partitions per core
W = 5000          # steady-state tile width
# Small leading tiles start ACT sooner; small trailing tiles shorten the
# serialized drain (last tile's DVE chain + store).
WIDTHS = (625, 1250, 2500) + (W,) * 8 + (2500, 1250, 1250, 625)
assert sum(WIDTHS) == T
EPS = 1e-6
SC = 32.0   # host-side input prescale
K = 5.0     # exp range shift

_CACHE: dict = {}

# Restrict Ln/Exp to the one table set that holds both, so the greedy
# ACT-table-load pass emits a single resident set (loaded once in the
# preamble) instead of alternating natural_log <-> exp_and_others.
_orig_gat = bacc.get_activation_tables


def _patched_gat(arch):
    A = mybir.ActivationFunctionType
    out = {}
    for name, fns in _orig_gat(arch).items():
        fns = set(fns)
        if name != "natural_log_exp_and_others":
            fns.discard(A.Ln)
            fns.discard(A.Exp)
        out[name] = fns
    return out


bacc.get_activation_tables = _patched_gat


def _poly2(r: float, delta: float):
    """Degree-2 Chebyshev fit of (R+delta)**r - delta**r on R in [0, 2]."""
    xs = np.linspace(0.0, 2.0, 200001)
    f = (xs + delta) ** r - delta ** r
    ch = np.polynomial.chebyshev.Chebyshev.fit(xs, f, 2, domain=[0.0, 2.0])
    c0, c1, c2 = ch.convert(kind=np.polynomial.Polynomial).coef
    return float(c0), float(c1), float(c2)


def _build(alpha: float, r: float, delta: float):
    f32 = mybir.dt.float32
    f16 = mybir.dt.float16
    Act = mybir.ActivationFunctionType
    Alu = mybir.AluOpType

    c0, c1, c2 = _poly2(r, delta)
    eK = float(np.exp(K))
    c2s = c2 * eK * eK          # poly in R' = R*e^-K
    a = c1 / (c2 * eK)          # u = R' + a
    exp_bias = -(K + float(np.log(SC)))

    nc = bacc.Bacc("TRN2", debug=False, enable_asserts=False,
                   target_bir_lowering=False)
    x = nc.dram_tensor("x", [FP, T], f16, kind="ExternalInput").ap()
    y = nc.dram_tensor("y", [FP, T], f16, kind="ExternalOutput").ap()

    with tile.TileContext(nc) as tc:
        with (
            tc.tile_pool(name="const", bufs=1) as cpool,
            tc.tile_pool(name="x", bufs=5) as xpool,
            tc.tile_pool(name="m", bufs=3) as mpool,
            tc.tile_pool(name="c", bufs=2) as carrypool,
            tc.tile_pool(name="e", bufs=3) as epool,
            tc.tile_pool(name="u", bufs=3) as upool,
        ):
            # stride-0 broadcast [FP,1] const for the scan's decay factor
            half = cpool.tile([FP, 1], f32, tag="half")
            nc.gpsimd.memset(half[:], 0.5)
            eps_t = cpool.tile([FP, 1], f32, tag="eps")
            nc.gpsimd.memset(eps_t[:], EPS)
            ebias_t = cpool.tile([FP, 1], f32, tag="ebias")
            nc.gpsimd.memset(ebias_t[:], exp_bias)
            # Warm-up activation with no data deps: the implicit first
            # ACT_TABLE_LOAD runs during the preamble instead of waiting
            # behind the first scan.
            warm = cpool.tile([FP, 1], f32, tag="warm")
            nc.scalar.activation(warm[:], eps_t[:], Act.Ln,
                                 bias=eps_t[:], scale=0.5)

            carry = 0.0
            off = 0
            for w in WIDTHS:
                xt = xpool.tile([FP, W], f16, tag="x")
                nc.sync.dma_start(xt[:, :w], x[:, off:off + w])
                m2 = mpool.tile([FP, W], f32, tag="m")
                nc.gpsimd.tensor_tensor_scan(
                    m2[:, :w], half[:].to_broadcast((FP, w)), xt[:, :w],
                    carry, Alu.mult, Alu.add)
                # Save the carry before Ln overwrites m2 in place; width-1
                # Pool op right after the scan, so the next scan never
                # waits on ACT and Ln never waits on the next scan.
                ct = carrypool.tile([FP, 1], f32, tag="c")
                nc.gpsimd.tensor_scalar_add(ct[:], m2[:, w - 1:w], 0.0)
                carry = ct[:]
                nc.scalar.activation(m2[:, :w], m2[:, :w], Act.Ln,
                                     bias=eps_t[:], scale=0.5 / SC)
                et = epool.tile([FP, W], f16, tag="e")
                nc.scalar.activation(et[:, :w], m2[:, :w], Act.Exp,
                                     scale=-alpha, bias=ebias_t[:])
                nc.vector.tensor_tensor(et[:, :w], xt[:, :w], et[:, :w],
                                        Alu.mult)           # R' = xs*e
                ut = upool.tile([FP, W], f16, tag="u")
                nc.vector.tensor_scalar_add(ut[:, :w], et[:, :w], a)
                nc.vector.tensor_tensor(ut[:, :w], ut[:, :w], et[:, :w],
                                        Alu.mult)           # u2 = u*R'
                nc.vector.tensor_scalar(ut[:, :w], ut[:, :w], c2s, c0,
                                        Alu.mult, Alu.add)
                nc.sync.dma_start(y[:, off:off + w], ut[:, :w])
                off += w

    nc.compile()
    return nc


def _get_nc(alpha: float, r: float, delta: float):
    key = (round(alpha, 9), round(r, 9), round(delta, 9))
    if key not in _CACHE:
        _CACHE[key] = _build(alpha, r, delta)
    return _CACHE[key]


def _make_runner(nc):
    """Cached variant of bass2jax.run_bass_via_pjrt's multi-core branch.

    run_bass_kernel_spmd builds a fresh jax.jit closure per call (full
    retrace) and round-trips the full array through per-core split +
    concat. Since the 8 shards concatenated on axis 0 ARE the full
    [1024, 50000] array, we jit once and feed/return the full array
    directly.
    """
    import jax
    from jax.experimental.shard_map import shard_map
    from jax.sharding import Mesh, PartitionSpec
    from concourse import bass2jax

    bass2jax.install_neuronx_cc_hook()
    if nc.dbg_callbacks:
        raise RuntimeError("dbg callbacks unsupported in cached runner")
    partition_name = (nc.partition_id_tensor.name
                      if nc.partition_id_tensor else None)
    in_names, out_names, out_avals = [], [], []
    for alloc in nc.m.functions[0].allocations:
        if not isinstance(alloc, mybir.MemoryLocationSet):
            continue
        name = alloc.memorylocations[0].name
        if alloc.kind == "ExternalInput":
            if name != partition_name:
                in_names.append(name)
        elif alloc.kind == "ExternalOutput":
            out_names.append(name)
            out_avals.append(jax.core.ShapedArray(
                tuple(alloc.tensor_shape), mybir.dt.np(alloc.dtype)))
    extra_ins = {}
    if nc.dbg_addr is not None:
        extra_ins[nc.dbg_addr.name] = np.zeros((1, 2), np.uint32)
        if nc.dbg_addr.name not in in_names:
            in_names.append(nc.dbg_addr.name)
    assert in_names[0] == "x" and out_names == ["y"], (in_names, out_names)
    n_params = len(in_names)
    all_names = list(in_names) + list(out_names)
    if partition_name is not None:
        all_names.append(partition_name)
    donate = tuple(range(n_params, n_params + len(out_names)))

    def _body(*args):
        operands = list(args)
        if partition_name is not None:
            operands.append(bass2jax.partition_id_tensor())
        outs = bass2jax._bass_exec_p.bind(
            *operands,
            out_avals=tuple(out_avals),
            in_names=tuple(all_names),
            out_names=tuple(out_names),
            lowering_input_output_aliases=(),
            sim_require_finite=True,
            sim_require_nnan=True,
            nc=nc,
        )
        return tuple(outs)

    devices = jax.devices()[:NCORES]
    assert len(devices) == NCORES, devices
    mesh = Mesh(np.asarray(devices), ("core",))
    nio = n_params + len(out_names)
    sharded = jax.jit(
        shard_map(_body, mesh=mesh,
                  in_specs=(PartitionSpec("core"),) * nio,
                  out_specs=(PartitionSpec("core"),) * len(out_names),
                  check_rep=False),
        donate_argnums=donate, keep_unused=True)

    def run(data16: np.ndarray) -> np.ndarray:
        extras = [np.concatenate([v] * NCORES, axis=0)
                  for v in extra_ins.values()]
        zeros = [np.zeros((NCORES * a.shape[0], *a.shape[1:]), a.dtype)
                 for a in out_avals]
        outs = sharded(data16, *extras, *zeros)
        return np.asarray(outs[0])

    return run


def kernel(data, alpha=None, r=None, delta=None) -> np.ndarray:
    data = np.asarray(data)
    assert data.shape == (F, T), data.shape
    a = float(np.asarray(alpha).reshape(-1)[0]) if alpha is not None else 0.98
    rr = float(np.asarray(r).reshape(-1)[0]) if r is not None else 0.5
    d = float(np.asarray(delta).reshape(-1)[0]) if delta is not None else 2.0

    data16 = np.ascontiguousarray(
        (data.astype(np.float32) * np.float32(SC)).astype(np.float16))

    nc = _get_nc(a, rr, d)
    rkey = ("runner", round(a, 9), round(rr, 9), round(d, 9))
    try:
        if rkey not in _CACHE:
            _CACHE[rkey] = _make_runner(nc)
        out16 = _CACHE[rkey](data16)
    except Exception:  # fall back to the stock SPMD path
        _CACHE[rkey] = None
        in_maps = [{"x": data16[i * FP:(i + 1) * FP]} for i in range(NCORES)]
        res = run_bass_kernel_spmd(nc, in_maps, core_ids=list(range(NCORES)))
        out16 = np.concatenate([res.results[i]["y"] for i in range(NCORES)],
                               axis=0)
    return out16.astype(np.float32)
